# revision 10
# baseline (speedup 1.0000x reference)
"""Batched OMP (DictionaryLearningTokenized) Trainium2 kernel.

Data-parallel over the pixel axis N=16384 across 8 NeuronCores (2048
pixels/core). Per core:
  - D^T -> DRAM (PE transpose) once; X^T tiles in SBUF (PE transpose) once
  - h_bar = X^T D per 128-pixel tile (PE matmul)
  - 8 OMP iterations:
      argmax |h| per pixel (ACT abs-evac from PSUM + DVE max8/max_index),
      per-tile indirect-DMA row gathers of selected atoms (D^T rows),
      Gram entries / h_bar entries computed as batched DVE dot products
        (gd = a.a + eps, gsel_j = a_j.a_s, hb_s = a_s.x),
      batched Cholesky update + triangular solves in a [pixel, tile] layout,
      h refresh via two accumulated PE matmuls: h = X^T D - recon^T D.
No cross-core communication; host shards/concatenates.
"""

import numpy as np
from contextlib import ExitStack

import concourse.bass as bass
import concourse.bacc as bacc
import concourse.tile as tile
from concourse import mybir
from concourse.bass_utils import run_bass_kernel_spmd

P = 128          # pixels per tile (SBUF partitions)
EMBED = 64       # feature dim d
K = 1024         # dictionary atoms
S = 8            # sparsity level
NCORES = 8
N_FULL = 16384
DIAG_EPS = 1e-4
CHOL_EPS = 1e-6

f32 = mybir.dt.float32
i32 = mybir.dt.int32
u32 = mybir.dt.uint32
AX = mybir.AxisListType
OP = mybir.AluOpType
AF = mybir.ActivationFunctionType


def _rowbase(i):  # packed lower-tri row-major block index of L(i, 0)
    return i * (i + 1) // 2


def _ltbase(i):  # packed lower-tri col-major block index of L(i, i)
    return i * S - i * (i - 1) // 2


def build_body(ctx: ExitStack, tc, io, T, debug=False):
    nc = tc.nc
    NPIX = P * T

    consts = ctx.enter_context(tc.tile_pool(name="consts", bufs=1))
    state = ctx.enter_context(tc.tile_pool(name="state", bufs=1))
    dram = ctx.enter_context(tc.tile_pool(name="dram", bufs=1, space="DRAM"))
    habs_pool = ctx.enter_context(tc.tile_pool(name="habsp", bufs=3))
    scan_pool = ctx.enter_context(tc.tile_pool(name="scanp", bufs=3))
    mrt_pool = ctx.enter_context(tc.tile_pool(name="mrtp", bufs=3))
    psum_h = ctx.enter_context(tc.tile_pool(name="psumh", bufs=3, space="PSUM"))
    psum_tr = ctx.enter_context(tc.tile_pool(name="psumtr", bufs=2, space="PSUM"))

    # ---- constants / inputs to SBUF
    ident = consts.tile([P, P], f32)
    nc.sync.dma_start(ident[:], io["IDENT"])
    D_sb = consts.tile([EMBED, K], f32)
    nc.sync.dma_start(D_sb[:], io["D"])
    X_sb = consts.tile([EMBED, NPIX], f32)
    nc.sync.dma_start(X_sb[:], io["X"])

    # ---- DRAM internals
    DT_d = dram.tile([K, EMBED], f32)

    # ---- persistent per-pixel state, layout [128 pixels, T tiles] per scalar
    Ls = state.tile([P, 36 * T], f32)      # L row-major packed
    LT = state.tile([P, 36 * T], f32)      # L col-major packed
    rL = state.tile([P, S * T], f32)       # 1/diag(L)
    ys = state.tile([P, S * T], f32)
    cs = state.tile([P, S * T], f32)
    hbs = state.tile([P, S * T], f32)      # h_bar at support
    gsel = state.tile([P, S * T], f32)     # G[I_j, idx_s] = a_j . a_s
    idx8 = state.tile([P, S * T * 8], u32)
    idxu = state.tile([P, S * T], u32)
    gd = state.tile([P, T], f32)
    sw = state.tile([P, T], f32)
    sw2 = state.tile([P, T], f32)
    prod = state.tile([P, S * T], f32)
    dprod = state.tile([P, T * EMBED], f32)  # dot-product scratch [P, T, 64]
    atoms = state.tile([P, S * T * EMBED], f32)
    XT = state.tile([P, T * EMBED], f32)     # x per pixel [P, T, 64]
    racc = state.tile([P, T * EMBED], f32)
    rtmp = state.tile([P, T * EMBED], f32)
    istage = state.tile([P, S * T], i32)

    def blk(ap, j, w=1):  # T-column block j (w blocks wide)
        return ap[:, j * T:(j + w) * T]

    def jt_view(ap, cnt):  # [P, cnt*T] j-major -> [P, T, cnt] for reduce over j
        return ap[:, :cnt * T].rearrange("p (j t) -> p t j", t=T)

    def idx_view(s):  # [P, T] uint32, stride 8 (top-1 of each tile's max8)
        return idx8[:, s * T * 8:(s + 1) * T * 8].rearrange(
            "p (t e) -> p t e", e=8)[:, :, 0]

    def v3(ap):  # [P, T*64] -> [P, T, 64]
        return ap.rearrange("p (t d) -> p t d", d=EMBED)

    def atoms_view(j):  # [P, T, 64]
        return v3(atoms[:, j * T * EMBED:(j + 1) * T * EMBED])

    def dot_all_tiles(out_PT, a3, b3):
        """out[p,t] = sum_d a3[p,t,:] * b3[p,t,:]  (two DVE ops)"""
        nc.vector.tensor_mul(v3(dprod[:]), a3, b3)
        nc.vector.tensor_reduce(out_PT, v3(dprod[:]), axis=AX.X, op=OP.add)

    # ---- D^T -> DRAM ; X^T tiles -> SBUF
    for b in range(S):
        tp = psum_tr.tile([P, EMBED], f32, tag="trp")
        nc.tensor.transpose(tp[:], D_sb[:, b * P:(b + 1) * P],
                            ident[:EMBED, :EMBED])
        tsb = mrt_pool.tile([P, EMBED], f32, tag="mrt")
        nc.scalar.activation(tsb[:], tp[:], AF.Copy)
        nc.sync.dma_start(DT_d[b * P:(b + 1) * P, :], tsb[:])
    for t in range(T):
        tp = psum_tr.tile([P, EMBED], f32, tag="trp")
        nc.tensor.transpose(tp[:], X_sb[:, t * P:(t + 1) * P],
                            ident[:EMBED, :EMBED])
        nc.scalar.activation(XT[:, t * EMBED:(t + 1) * EMBED], tp[:], AF.Copy)

    def scan_tile(t, hp, s):
        """abs-evac h from PSUM, top-8 + indices -> idx8 block (s, t)."""
        habs = habs_pool.tile([P, K], f32, tag="habs")
        nc.scalar.activation(habs[:], hp[:], AF.Abs)
        t8 = scan_pool.tile([P, 8], f32, tag="t8")
        nc.vector.max(out=t8[:], in_=habs[:])
        nc.vector.max_index(
            out=idx8[:, (s * T + t) * 8:(s * T + t + 1) * 8],
            in_max=t8[:], in_values=habs[:])

    # ---- h_bar phase (scan for iteration 0)
    for t in range(T):
        hp = psum_h.tile([P, K], f32, tag="hps")
        for h in range(2):
            nc.tensor.matmul(
                out=hp[:, h * 512:(h + 1) * 512],
                lhsT=X_sb[:, t * P:(t + 1) * P],
                rhs=D_sb[:, h * 512:(h + 1) * 512],
                start=True, stop=True)
        scan_tile(t, hp, 0)

    # ---- OMP iterations
    for s in range(S):
        rb = _rowbase(s)
        # contiguous copy of this iteration's indices
        nc.vector.tensor_copy(blk(idxu, s), idx_view(s))
        # per-tile atom row gathers: atoms_s[:, t, :] = DT[idx[:, t], :]
        for t in range(T):
            nc.gpsimd.indirect_dma_start(
                out=atoms[:, (s * T + t) * EMBED:(s * T + t + 1) * EMBED],
                out_offset=None,
                in_=DT_d[:],
                in_offset=bass.IndirectOffsetOnAxis(
                    ap=idxu[:, s * T + t:s * T + t + 1], axis=0))
        a_s = atoms_view(s)
        # gd = a_s . a_s + DIAG_EPS
        dot_all_tiles(gd[:], a_s, a_s)
        nc.vector.tensor_scalar(gd[:], gd[:], DIAG_EPS, None, op0=OP.add)
        # hbs_s = a_s . x
        dot_all_tiles(blk(hbs, s), a_s, v3(XT[:]))

        if s > 0:
            # gsel_j = a_j . a_s for j < s
            for j in range(s):
                dot_all_tiles(blk(gsel, j), atoms_view(j), a_s)
            # Cholesky new row: solve L w = gsel (forward), w -> row s of L
            for i in range(s):
                if i == 0:
                    nc.vector.tensor_mul(blk(Ls, rb + 0), blk(gsel, 0),
                                         blk(rL, 0))
                else:
                    pr = prod[:, :i * T]
                    nc.vector.tensor_mul(pr, Ls[:, _rowbase(i) * T:
                                                (_rowbase(i) + i) * T],
                                         blk(Ls, rb, i))
                    nc.vector.tensor_reduce(sw[:], jt_view(prod, i),
                                            axis=AX.X, op=OP.add)
                    nc.vector.tensor_sub(sw2[:], blk(gsel, i), sw[:])
                    nc.vector.tensor_mul(blk(Ls, rb + i), sw2[:], blk(rL, i))
                nc.vector.tensor_copy(blk(LT, _ltbase(i) + s - i),
                                      blk(Ls, rb + i))
            # corner = sqrt(clip(gd - sum w^2))
            pr = prod[:, :s * T]
            nc.vector.tensor_mul(pr, blk(Ls, rb, s), blk(Ls, rb, s))
            nc.vector.tensor_reduce(sw[:], jt_view(prod, s), axis=AX.X,
                                    op=OP.add)
            nc.vector.tensor_sub(sw2[:], gd[:], sw[:])
            nc.vector.tensor_scalar(sw2[:], sw2[:], CHOL_EPS, None, op0=OP.max)
        else:
            nc.vector.tensor_scalar(sw2[:], gd[:], CHOL_EPS, None, op0=OP.max)
        nc.scalar.sqrt(blk(Ls, rb + s), sw2[:])
        nc.vector.tensor_copy(blk(LT, _ltbase(s)), blk(Ls, rb + s))
        nc.vector.reciprocal(blk(rL, s), blk(Ls, rb + s))

        # forward: y_s = (hbs_s - sum_{j<s} L(s,j) y_j) / L(s,s)
        if s == 0:
            nc.vector.tensor_mul(blk(ys, 0), blk(hbs, 0), blk(rL, 0))
        else:
            pr = prod[:, :s * T]
            nc.vector.tensor_mul(pr, blk(Ls, rb, s), ys[:, :s * T])
            nc.vector.tensor_reduce(sw[:], jt_view(prod, s), axis=AX.X,
                                    op=OP.add)
            nc.vector.tensor_sub(sw2[:], blk(hbs, s), sw[:])
            nc.vector.tensor_mul(blk(ys, s), sw2[:], blk(rL, s))

        # backward: x_i = (y_i - sum_{j>i} L(j,i) x_j) / L(i,i)
        nc.vector.tensor_mul(blk(cs, s), blk(ys, s), blk(rL, s))
        for i in range(s - 1, -1, -1):
            cnt = s - i
            pr = prod[:, :cnt * T]
            nc.vector.tensor_mul(pr, blk(LT, _ltbase(i) + 1, cnt),
                                 cs[:, (i + 1) * T:(s + 1) * T])
            nc.vector.tensor_reduce(sw[:], jt_view(prod, cnt), axis=AX.X,
                                    op=OP.add)
            nc.vector.tensor_sub(sw2[:], blk(ys, i), sw[:])
            nc.vector.tensor_mul(blk(cs, i), sw2[:], blk(rL, i))

        # recon = sum_j c_j * atoms_j   [P, T, 64]
        nc.vector.tensor_mul(v3(racc[:]), atoms_view(0),
                             blk(cs, 0).to_broadcast([P, T, EMBED]))
        for j in range(1, s + 1):
            nc.vector.tensor_mul(v3(rtmp[:]), atoms_view(j),
                                 blk(cs, j).to_broadcast([P, T, EMBED]))
            nc.vector.tensor_add(racc[:], racc[:], rtmp[:])

        if s < S - 1:
            # h = X^T D - recon^T D per tile; evac |h| and scan for iter s+1
            for t in range(T):
                tp = psum_tr.tile([EMBED, P], f32, tag="trp")
                nc.tensor.transpose(tp[:], racc[:, t * EMBED:(t + 1) * EMBED],
                                    ident[:])
                mrt = mrt_pool.tile([EMBED, P], f32, tag="mrt")
                nc.scalar.activation(mrt[:], tp[:], AF.Copy, scale=-1.0)
                hp = psum_h.tile([P, K], f32, tag="hps")
                for h in range(2):
                    nc.tensor.matmul(
                        out=hp[:, h * 512:(h + 1) * 512],
                        lhsT=X_sb[:, t * P:(t + 1) * P],
                        rhs=D_sb[:, h * 512:(h + 1) * 512],
                        start=True, stop=False)
                    nc.tensor.matmul(
                        out=hp[:, h * 512:(h + 1) * 512],
                        lhsT=mrt[:],
                        rhs=D_sb[:, h * 512:(h + 1) * 512],
                        start=False, stop=True)
                scan_tile(t, hp, s + 1)
        else:
            if debug:
                nc.sync.dma_start(io["dbg_idx8"], idx8[:])
                nc.sync.dma_start(io["dbg_hbs"], hbs[:])
                nc.sync.dma_start(io["dbg_gsel"], gsel[:, :7 * T])
                nc.sync.dma_start(io["dbg_atoms"], atoms[:])
                nc.sync.dma_start(io["dbg_Ls"], Ls[:])
                nc.sync.dma_start(io["dbg_ys"], ys[:])
                nc.sync.dma_start(io["dbg_gd"], gd[:])
            # outputs
            nc.sync.dma_start(
                io["recon"].rearrange("(t p) d -> p t d", p=P),
                v3(racc[:]))
            nc.vector.tensor_copy(istage[:], idxu[:])
            nc.sync.dma_start(
                io["I"].rearrange("(t p) s -> p s t", p=P),
                istage.rearrange("p (s t) -> p s t", t=T))
            nc.sync.dma_start(
                io["coeffs"].rearrange("(t p) s -> p s t", p=P),
                cs.rearrange("p (s t) -> p s t", t=T))


def build_nc(T=16, debug=False):
    NPIX = P * T
    nc = bacc.Bacc("TRN2", debug=False)
    io = {
        "X": nc.dram_tensor("X", [EMBED, NPIX], f32, kind="ExternalInput").ap(),
        "D": nc.dram_tensor("D", [EMBED, K], f32, kind="ExternalInput").ap(),
        "IDENT": nc.dram_tensor("IDENT", [P, P], f32,
                                kind="ExternalInput").ap(),
        "recon": nc.dram_tensor("recon", [NPIX, EMBED], f32,
                                kind="ExternalOutput").ap(),
        "I": nc.dram_tensor("I", [NPIX, S], i32, kind="ExternalOutput").ap(),
        "coeffs": nc.dram_tensor("coeffs", [NPIX, S], f32,
                                 kind="ExternalOutput").ap(),
    }
    if debug:
        for nm, shp, dt in [("dbg_idx8", [P, S * T * 8], u32),
                            ("dbg_hbs", [P, S * T], f32),
                            ("dbg_gsel", [P, 7 * T], f32),
                            ("dbg_atoms", [P, S * T * EMBED], f32),
                            ("dbg_Ls", [P, 36 * T], f32),
                            ("dbg_ys", [P, S * T], f32),
                            ("dbg_gd", [P, T], f32)]:
            io[nm] = nc.dram_tensor(nm, shp, dt, kind="ExternalOutput").ap()
    with tile.TileContext(nc) as tc:
        with ExitStack() as ctx:
            build_body(ctx, tc, io, T, debug=debug)
    nc.compile()
    return nc


def make_consts(T):
    return {
        "IDENT": np.eye(P, dtype=np.float32),
    }


_cache = {}


def kernel(X, D):
    X = np.ascontiguousarray(np.asarray(X, dtype=np.float32))
    D = np.ascontiguousarray(np.asarray(D, dtype=np.float32))
    T = N_FULL // NCORES // P
    if "nc" not in _cache:
        _cache["nc"] = build_nc(T)
    nc = _cache["nc"]
    consts = make_consts(T)
    npc = N_FULL // NCORES
    in_maps = [
        {"X": np.ascontiguousarray(X[:, c * npc:(c + 1) * npc]),
         "D": D, **consts}
        for c in range(NCORES)
    ]
    res = run_bass_kernel_spmd(nc, in_maps, core_ids=list(range(NCORES)))
    results = res.results
    recon = np.concatenate([results[c]["recon"] for c in range(NCORES)], 0)
    I = np.concatenate([results[c]["I"] for c in range(NCORES)], 0)
    C = np.concatenate([results[c]["coeffs"] for c in range(NCORES)], 0)
    return recon, I.astype(np.int32), C
